# revision 3
# baseline (speedup 1.0000x reference)
"""Cross-attention kernel for 8 trn2 NeuronCores.

Reference computation (per batch b of 16):
  q = Wq @ x, k = Wk @ y, v = Wv @ y          (1x1 convs as channel matmuls)
  q,k l2-normalized over the SPATIAL axis (per (h,d) row)
  sim = 10 * q^T k per head; attn = softmax_j(sim); o = attn @ v^T
  out = Wo @ o + b

Sharding: data-parallel over batch, 2 batches per core, weights replicated.

Per-core kernel layout choices:
  - q,k projections and the q^T k similarity run in fp8(e4m3) with the
    DoubleRow perf mode: contraction is split into two partition-planes
    processed per cycle, so a projection chunk is ONE matmul (both C-halves)
    and sim runs at 0.5 cycles/row.  q,k live as [128, 2, N] fp8 tiles with
    partition = 32*head + (d%32), plane = d//32; the weight columns are
    permuted host-side to produce that layout directly.
  - the combined l2-norm scale 1/(||q_d||*||k_d||) (computed from bn_stats
    on the fp32 PSUM projections + a Quake rsqrt on DVE) is folded into k
    together with the *10 softmax scale and a *128 range boost that keeps
    the fp8 k values ~N(0,1.25); the exp() then uses scale 2^-7.
  - v, PV, and the output projection stay fp16 (fp8 there would put ~3%
    noise straight onto the output; the q/k path only perturbs logits by
    ~0.003 because softmax logits are small for these inputs).
  - sim computed TRANSPOSED: S_T[j, i] = sum_d k[d,j] q[d,i]; softmax
    denominator = row 64 of the PV accumulation via an all-ones column
    appended to v^T.  exp needs no max-subtraction (|logit| <= 0.7).
  - normalization: denominator row reshaped [1,1024]->[128,8] via SBUF DMA,
    reciprocal on DVE, reshaped back, partition_broadcast on GPSIMD, then
    one tensor_tensor multiply PSUM->SBUF.
"""

import sys

import numpy as np

if "/opt/trn_rl_repo" not in sys.path:
    sys.path.insert(0, "/opt/trn_rl_repo")

NB = 2        # batches per core
C = 256       # channels
N = 1024      # spatial (32*32)
HEADS = 4
DH = 64
HID = 256
NCORES = 8
MAGIC = 0x5F3759DF  # Quake fast inverse-sqrt seed
KSCALE = 640.0 / N    # rsqrt(uq*uk)*10*128/(2N): sim double-counted by the stride-0 DR planes
ESCALE = 1.0 / 128.0  # exp(st * 2^-7)

_CACHE = {}


def _quake_rsqrt(nc, pool, p_ap, out_ap, final_scale):
    """out = rsqrt(p) * final_scale for [128,1] fp32 APs, DVE-only.

    Quake seed + 2 Newton iterations (rel err ~1e-7), no ACT table needed.
    """
    from concourse import mybir

    i32 = mybir.dt.int32
    alu = mybir.AluOpType
    t = pool.tile([128, 1], mybir.dt.float32, tag="qk_rs_t", bufs=4)
    r = pool.tile([128, 1], mybir.dt.float32, tag="qk_rs_r", bufs=4)
    a = pool.tile([128, 1], mybir.dt.float32, tag="qk_rs_a", bufs=4)
    # seed: r0 = bitcast(MAGIC - (bitcast_i32(p) >> 1))
    nc.vector.tensor_scalar(t.bitcast(i32), p_ap.bitcast(i32), 1, None,
                            alu.logical_shift_right)
    nc.vector.tensor_scalar(r.bitcast(i32), t.bitcast(i32), -1, MAGIC,
                            alu.mult, alu.add)
    # Newton 1: r = r * (1.5 - 0.5 * p * r^2)
    nc.vector.scalar_tensor_tensor(a[:], r[:], r[:, 0:1], p_ap,
                                   alu.mult, alu.mult)
    nc.vector.tensor_scalar(a[:], a[:], -0.5, 1.5, alu.mult, alu.add)
    nc.vector.tensor_scalar(t[:], a[:], r[:, 0:1], None, alu.mult)
    # Newton 2 (fold final_scale into the last multiply)
    nc.vector.scalar_tensor_tensor(a[:], t[:], t[:, 0:1], p_ap,
                                   alu.mult, alu.mult)
    nc.vector.tensor_scalar(a[:], a[:], -0.5, 1.5, alu.mult, alu.add)
    nc.vector.tensor_scalar(out_ap, a[:], t[:, 0:1], final_scale,
                            alu.mult, alu.mult)


def _build_nc():
    from contextlib import ExitStack

    import concourse.tile as tile
    from concourse import bacc, mybir

    f32 = mybir.dt.float32
    f16 = mybir.dt.float16
    f8 = mybir.dt.float8e4
    alu = mybir.AluOpType
    EXP = mybir.ActivationFunctionType.Exp
    DR = mybir.MatmulPerfMode.DoubleRow

    nc = bacc.Bacc("TRN2", target_bir_lowering=False)

    x8in = nc.dram_tensor("x8", [NB, C, N], f8, kind="ExternalInput")
    y8in = nc.dram_tensor("y8", [NB, C, N], f8, kind="ExternalInput")
    y16in = nc.dram_tensor("y16", [NB, C, N], f16, kind="ExternalInput")
    wq = nc.dram_tensor("wq_t", [C, HID], f8, kind="ExternalInput")
    wk = nc.dram_tensor("wk_t", [C, HID], f8, kind="ExternalInput")
    wv = nc.dram_tensor("wv_t", [C, HID], f16, kind="ExternalInput")
    wo = nc.dram_tensor("wo_t", [HID, C], f16, kind="ExternalInput")
    bo = nc.dram_tensor("b_out", [2, 128, 1], f32, kind="ExternalInput")
    out = nc.dram_tensor("out", [NB, C, N], f32, kind="ExternalOutput")

    with tile.TileContext(nc) as tc, ExitStack() as ctx:
        consts = ctx.enter_context(tc.tile_pool(name="consts", bufs=1))
        big = ctx.enter_context(tc.tile_pool(name="big", bufs=2))
        sm = ctx.enter_context(tc.tile_pool(name="sm", bufs=4))
        ps = ctx.enter_context(tc.tile_pool(name="ps", bufs=2, space="PSUM"))

        # ---- input + weight loads (batch-0 x/y first: critical path) --
        wq_sb = consts.tile([128, 2, HID], f8, tag="wq")
        wk_sb = consts.tile([128, 2, HID], f8, tag="wk")
        wv_sb = consts.tile([128, 2, HID], f16, tag="wv")
        wo_sb = consts.tile([128, 2, C], f16, tag="wo")
        b_sb = consts.tile([128, 2, 1], f32, tag="bo")
        # warm the ACT exp table while input DMAs are in flight
        warm = sm.tile([128, 1], f32, tag="warm", bufs=1)
        nc.vector.memset(warm[:], 0.0)
        nc.scalar.activation(out=warm[:], in_=warm[:], func=EXP, scale=1.0)
        xts, yts, y16s = [], [], []
        for nb in range(NB):
            xt = big.tile([128, 2, N], f8, tag="xt", bufs=2)
            yt = big.tile([128, 2, N], f8, tag="yt", bufs=2)
            y16 = big.tile([128, 2, N], f16, tag="y16", bufs=2)
            xts.append(xt)
            yts.append(yt)
            y16s.append(y16)
        nc.sync.dma_start(out=yts[0][:], in_=y8in[0].rearrange("(kc p) n -> p kc n", p=128))
        nc.sync.dma_start(out=xts[0][:], in_=x8in[0].rearrange("(kc p) n -> p kc n", p=128))
        nc.sync.dma_start(out=wk_sb[:], in_=wk.rearrange("(kc p) n -> p kc n", p=128))
        nc.sync.dma_start(out=wq_sb[:], in_=wq.rearrange("(kc p) n -> p kc n", p=128))
        nc.sync.dma_start(out=y16s[0][:], in_=y16in[0].rearrange("(kc p) n -> p kc n", p=128))
        nc.sync.dma_start(out=wv_sb[:], in_=wv.rearrange("(kc p) n -> p kc n", p=128))
        nc.sync.dma_start(out=wo_sb[:], in_=wo.rearrange("(kc p) n -> p kc n", p=128))
        nc.sync.dma_start(out=b_sb[:], in_=bo.rearrange("kc p n -> p kc n"))
        nc.sync.dma_start(out=xts[1][:], in_=x8in[1].rearrange("(kc p) n -> p kc n", p=128))
        nc.sync.dma_start(out=yts[1][:], in_=y8in[1].rearrange("(kc p) n -> p kc n", p=128))
        nc.sync.dma_start(out=y16s[1][:], in_=y16in[1].rearrange("(kc p) n -> p kc n", p=128))

        # ---- per-batch stages --------------------------------------
        def proj_qk(nb, qn, kn, c2):
            """Head-pair chunk c2 of the q,k projections (fp8, DoubleRow).

            One matmul contracts both C-halves (planes of x8/y8); psum
            partition = 64*(h%2) + d, chunk c2 = h//2 (baseline layout).
            """
            kp = ps.tile([128, N], f32, tag="ps_acc", bufs=2)
            qp = ps.tile([128, N], f32, tag="ps_acc", bufs=2)
            for ih in range(2):
                nc.tensor.matmul(
                    kp[:, ih * 512:(ih + 1) * 512],
                    wk_sb[:, :, c2 * 128:(c2 + 1) * 128],
                    yts[nb][:, :, ih * 512:(ih + 1) * 512],
                    start=True, stop=True, perf_mode=DR)
            for ih in range(2):
                nc.tensor.matmul(
                    qp[:, ih * 512:(ih + 1) * 512],
                    wq_sb[:, :, c2 * 128:(c2 + 1) * 128],
                    xts[nb][:, :, ih * 512:(ih + 1) * 512],
                    start=True, stop=True, perf_mode=DR)
            # q is copied raw to fp8; the combined norm scale goes on k
            nc.vector.tensor_copy(qn[:, c2, :], qp[:])
            stq = sm.tile([128, 2, 6], f32, tag="stq", bufs=4)
            stk = sm.tile([128, 2, 6], f32, tag="stk", bufs=4)
            mvq = sm.tile([128, 2], f32, tag="mvq", bufs=4)
            mvk = sm.tile([128, 2], f32, tag="mvk", bufs=4)
            for sub in range(2):
                nc.vector.bn_stats(out=stk[:, sub, :], in_=kp[:, sub * 512:(sub + 1) * 512])
                nc.vector.bn_stats(out=stq[:, sub, :], in_=qp[:, sub * 512:(sub + 1) * 512])
            nc.vector.bn_aggr(out=mvk[:], in_=stk[:])
            nc.vector.bn_aggr(out=mvq[:], in_=stq[:])
            uq = sm.tile([128, 1], f32, tag="uq", bufs=4)
            uk = sm.tile([128, 1], f32, tag="uk", bufs=4)
            pqk = sm.tile([128, 1], f32, tag="pqk", bufs=4)
            nc.vector.scalar_tensor_tensor(uq[:], mvq[:, 0:1], mvq[:, 0:1],
                                           mvq[:, 1:2], alu.mult, alu.add)
            nc.vector.scalar_tensor_tensor(uk[:], mvk[:, 0:1], mvk[:, 0:1],
                                           mvk[:, 1:2], alu.mult, alu.add)
            nc.vector.tensor_mul(pqk[:], uq[:], uk[:])
            sck = sm.tile([128, 1], f32, tag="sck", bufs=4)
            _quake_rsqrt(nc, sm, pqk[:], sck[:], KSCALE)
            nc.vector.tensor_scalar(kn[:, c2, :], kp[:], sck[:, 0:1], None, alu.mult)

        def proj_v(nb, jcs, vts_nb):
            for jc in jcs:
                vp = ps.tile([128, N], f32, tag="ps_acc", bufs=2)
                for kc in range(2):
                    nc.tensor.matmul(
                        vp[:, 0:HID],
                        y16s[nb][:, kc, jc * 128:(jc + 1) * 128],
                        wv_sb[:, kc, :],
                        start=(kc == 0), stop=(kc == 1))
                vt = big.tile([128, 4, 65], f16, tag="vt", bufs=16)
                nc.vector.tensor_copy(vt[:, :, 0:64],
                                      vp[:, 0:HID].rearrange("p (h d) -> p h d", h=4))
                nc.gpsimd.memset(vt[:, :, 64:65], 1.0)
                vts_nb.append(vt)

        def attn_head(nb, h, qn, kn, vts_nb, o_sb):
            hp, hr = h // 2, 64 * (h % 2)
            op = ps.tile([128, N], f32, tag="ps_acc", bufs=2)
            for jc in range(8):
                st = ps.tile([128, N], f32, tag="ps_big", bufs=2)
                for ih in range(2):
                    # stride-0 "plane" dim: DR reads the 64-d contraction
                    # twice (result 2x, folded into KSCALE)
                    nc.tensor.matmul(
                        st[:, ih * 512:(ih + 1) * 512],
                        kn[hr:hr + 64, hp, jc * 128:(jc + 1) * 128]
                        .unsqueeze(1).broadcast_to([64, 2, 128]),
                        qn[hr:hr + 64, hp, ih * 512:(ih + 1) * 512]
                        .unsqueeze(1).broadcast_to([64, 2, 512]),
                        start=True, stop=True, perf_mode=DR)
                et = big.tile([128, N], f16, tag="et", bufs=8)
                nc.scalar.activation(out=et[:], in_=st[:], func=EXP, scale=ESCALE)
                vt = vts_nb[jc]
                for ih in range(2):
                    nc.tensor.matmul(
                        op[0:65, ih * 512:(ih + 1) * 512],
                        vt[:, h, :],
                        et[:, ih * 512:(ih + 1) * 512],
                        start=(jc == 0), stop=(jc == 7))
            # normalization: row 64 of op is the softmax denominator
            srow = sm.tile([1, N], f32, tag="srow", bufs=4)
            s2 = sm.tile([128, 8], f32, tag="s2", bufs=4)
            r2 = sm.tile([128, 8], f32, tag="r2", bufs=4)
            rrow = sm.tile([1, N], f32, tag="rrow", bufs=4)
            rb = big.tile([64, N], f32, tag="rb", bufs=4)
            nc.vector.tensor_copy(srow[:], op[64:65, :])
            nc.sync.dma_start(out=s2[:], in_=srow[:])
            nc.vector.reciprocal(r2[:], s2[:])
            nc.sync.dma_start(out=rrow[:], in_=r2[:])
            nc.gpsimd.partition_broadcast(rb[:], rrow[:])
            nc.vector.tensor_mul(o_sb[hr:hr + 64, hp, :], op[0:64, :], rb[:])

        def zproj(nb, o_sb):
            for mc in range(2):
                zp = ps.tile([128, N], f32, tag="ps_acc", bufs=2)
                for ih in range(2):
                    for kc in range(2):
                        nc.tensor.matmul(
                            zp[:, ih * 512:(ih + 1) * 512],
                            wo_sb[:, kc, mc * 128:(mc + 1) * 128],
                            o_sb[:, kc, ih * 512:(ih + 1) * 512],
                            start=(kc == 0), stop=(kc == 1))
                zs = big.tile([128, N], f32, tag="zs", bufs=4)
                nc.vector.tensor_scalar(zs[:], zp[:], b_sb[:, mc, 0:1], None, alu.add)
                nc.sync.dma_start(out=out[nb, mc * 128:(mc + 1) * 128, :], in_=zs[:])

        def alloc_qk():
            qn = big.tile([128, 2, N], f8, tag="qn", bufs=2)
            kn = big.tile([128, 2, N], f8, tag="kn", bufs=2)
            return qn, kn

        def alloc_o():
            o_sb = big.tile([128, 2, N], f16, tag="osb", bufs=2)
            return o_sb

        qn0, kn0 = alloc_qk()
        vts0 = []
        proj_qk(0, qn0, kn0, 0)
        proj_qk(0, qn0, kn0, 1)
        proj_v(0, range(8), vts0)
        o0 = alloc_o()
        qn1, kn1 = alloc_qk()
        vts1 = []
        attn_head(0, 0, qn0, kn0, vts0, o0)
        proj_qk(1, qn1, kn1, 0)
        attn_head(0, 1, qn0, kn0, vts0, o0)
        proj_qk(1, qn1, kn1, 1)
        attn_head(0, 2, qn0, kn0, vts0, o0)
        proj_v(1, range(0, 4), vts1)
        attn_head(0, 3, qn0, kn0, vts0, o0)
        o1 = alloc_o()
        proj_v(1, range(4, 8), vts1)
        attn_head(1, 0, qn1, kn1, vts1, o1)
        zproj(0, o0)
        attn_head(1, 1, qn1, kn1, vts1, o1)
        attn_head(1, 2, qn1, kn1, vts1, o1)
        attn_head(1, 3, qn1, kn1, vts1, o1)
        zproj(1, o1)

    nc.finalize()
    return nc


def _get_nc():
    if "nc" not in _CACHE:
        _CACHE["nc"] = _build_nc()
    return _CACHE["nc"]


def kernel(x, y, w_qkv, w_out, b_out):
    import ml_dtypes
    from concourse.bass_utils import run_bass_kernel_spmd

    nc = _get_nc()
    f8 = ml_dtypes.float8_e4m3

    x = np.asarray(x, dtype=np.float32).reshape(16, C, N)
    y = np.asarray(y, dtype=np.float32).reshape(16, C, N)
    x8 = x.astype(f8)
    y8 = y.astype(f8)
    y16 = y.astype(np.float16)
    w_qkv = np.asarray(w_qkv, dtype=np.float32)
    wq_t = np.ascontiguousarray(w_qkv[0:HID].T).astype(f8)
    wk_t = np.ascontiguousarray(w_qkv[HID:2 * HID].T).astype(f8)
    wv_t = np.ascontiguousarray(w_qkv[2 * HID:3 * HID].T).astype(np.float16)
    wo_t = np.ascontiguousarray(np.asarray(w_out, dtype=np.float32).T).astype(np.float16)
    bo = np.ascontiguousarray(
        np.asarray(b_out, dtype=np.float32).reshape(2, 128, 1))

    in_maps = []
    for c in range(NCORES):
        in_maps.append({
            "x8": np.ascontiguousarray(x8[c * NB:(c + 1) * NB]),
            "y8": np.ascontiguousarray(y8[c * NB:(c + 1) * NB]),
            "y16": np.ascontiguousarray(y16[c * NB:(c + 1) * NB]),
            "wq_t": wq_t, "wk_t": wk_t, "wv_t": wv_t, "wo_t": wo_t,
            "b_out": bo,
        })

    res = run_bass_kernel_spmd(nc, in_maps, list(range(NCORES)))
    full = np.concatenate([res.results[i]["out"] for i in range(NCORES)], axis=0)
    return full.reshape(16, C, 32, 32)
